# revision 1
# baseline (speedup 1.0000x reference)
"""Causal multi-head self-attention on 8 TRN2 NeuronCores.

Problem (hardcoded): x (4, 2048, 1024) f32, W_qkv (3072, 1024), W_o (1024, 1024).
  qkv = x @ W_qkv.T; q,k,v split -> (B,H,T,DK) with H=16, DK=64
  scores = q k^T / 8 + causal mask; attn = softmax; out = (attn v) @ W_o.T

Sharding: core = 2*b + hg  (b in 0..3 batches, hg in 0..1 head-groups of 8 heads).
Each core computes a partial out[b] over its 512 attn columns; host sums pairs.

Per-core dataflow ("T-attention": t2-on-partitions S^T tiles):
  - qT,kT [e=512, T] via f32r matmuls; v in t-major [t, h, dk] with an extra
    ones column so the AV matmul also accumulates softmax denominators.
  - S^T tile [t2:128, t1<=512] = kT_h^T-slice x qT_h-slice (K=DK=64, bf16);
    causal handled by skipping fully-masked column ranges and one [128,128]
    triangle mask add on the diagonal block.
  - exp via ACT with fused 1/8 scale (no max-subtraction; |scores|/8 <~ 2).
  - AV: psum[65, 512] += v_aug^T-slice x P^T tile (row 64 = row sums l).
    AV pairs are emitted one pair behind the S/exp stream so the PE never
    waits on the ACT exp (lag-1 software pipeline).
  - normalize: early-evacuate psum to SBUF (frees the bank), reciprocal of l
    on DVE, partition-broadcast via a K=1 PE matmul, multiply on DVE; emitted
    one head late (lag-1) so the PE rarely waits on the reciprocal.
  - out_proj: attn^T x W_o-slice, f32 partial to DRAM.
"""

import sys

import numpy as np

sys.path.insert(0, "/opt/trn_rl_repo")

import ml_dtypes  # noqa: E402

from concourse import bacc, bass, mybir, tile  # noqa: E402
from concourse.bass_utils import run_bass_kernel_spmd  # noqa: E402

FP32 = mybir.dt.float32
FP32R = mybir.dt.float32r
BF16 = mybir.dt.bfloat16

B, T, D, H, DK = 4, 2048, 1024, 16, 64
NCORES = 8
E = 512          # qkv columns per head-group
NH = 8           # heads per core
P = 128
DCH = D // P     # 8 contraction chunks for the projections
EC = E // P      # 4 e-chunks for q/k
NT512 = T // 512
NT128 = T // P

MASK_VAL = -1e9


def _emit(nc, tc, ctx, xT, wq, wk, wv, wo, mtri, sel, out):
    from contextlib import ExitStack as _ES  # noqa: F401

    consts = ctx.enter_context(tc.tile_pool(name="consts", bufs=1))
    persist = ctx.enter_context(tc.tile_pool(name="persist", bufs=1))

    # Persistent SBUF state
    qT = persist.tile([P, EC, T], BF16, name="qT")        # e = ec*128+p
    kT = persist.tile([P, EC, T], BF16, name="kT")
    vt = persist.tile([P, NT128, NH, DK + NH], BF16, name="vt")  # + one-hot cols
    attn = persist.tile([P, EC, T], BF16, name="attn")    # dl = s*128+p
    wo_sb = persist.tile([P, EC, D], BF16, name="wo_sb")  # dl = s*128+p
    mtri_sb = consts.tile([P, P], FP32, name="mtri_sb")
    sel_sb = consts.tile([NH, NH * DK], FP32R, name="sel_sb")

    nc.sync.dma_start(mtri_sb[:], mtri[:])
    nc.sync.dma_start(sel_sb[:], sel[:])
    # head h's ones column sits at DK+h so its denominators land on a
    # distinct psum partition (64+h); other heads' columns there are zero
    nc.vector.memset(vt[:, :, :, DK:], 0.0)
    for hh in range(NH):
        nc.vector.memset(vt[:, :, hh, DK + hh], 1.0)

    # ---- Phase B: QKV projections (xT and weight tiles freed afterwards) ----
    with (
        tc.tile_pool(name="xw", bufs=1) as xw,
        tc.tile_pool(name="pb", bufs=2, space="PSUM") as pb,
    ):
        xT_sb = xw.tile([P, DCH, T], BF16, name="xT_sb")
        wq_sb = xw.tile([P, DCH, E], BF16, name="wq_sb")
        wk_sb = xw.tile([P, DCH, E], BF16, name="wk_sb")
        wv_sb = xw.tile([P, DCH, E], BF16, name="wv_sb")
        # q-projection inputs first so the PE starts as soon as possible;
        # wk/wv stream behind them, wo last (first needed ~150us in)
        for j in range(DCH):
            nc.sync.dma_start(xT_sb[:, j], xT[j * P : (j + 1) * P, :])
            nc.sync.dma_start(wq_sb[:, j], wq[j * P : (j + 1) * P, :])
        for j in range(DCH):
            nc.sync.dma_start(wk_sb[:, j], wk[j * P : (j + 1) * P, :])
        for j in range(DCH):
            nc.sync.dma_start(wv_sb[:, j], wv[j * P : (j + 1) * P, :])
        for s in range(EC):
            nc.sync.dma_start(wo_sb[:, s], wo[s * P : (s + 1) * P, :])

        for w_sb, dst in ((wq_sb, qT), (wk_sb, kT)):
            for ec in range(EC):
                for c in range(NT512):
                    ps = pb.tile([P, 512], FP32, name="ps_qk")
                    for j in range(DCH):
                        nc.tensor.matmul(
                            ps[:],
                            lhsT=w_sb[:, j, ec * P : (ec + 1) * P],
                            rhs=xT_sb[:, j, c * 512 : (c + 1) * 512],
                            start=(j == 0),
                            stop=(j == DCH - 1),
                        )
                    nc.vector.tensor_copy(dst[:, ec, c * 512 : (c + 1) * 512], ps[:])

        for t16 in range(NT128):
            psv = pb.tile([P, NH, DK], FP32, name="ps_v")
            for j in range(DCH):
                nc.tensor.matmul(
                    psv[:],
                    lhsT=xT_sb[:, j, t16 * P : (t16 + 1) * P],
                    rhs=wv_sb[:, j, :],
                    start=(j == 0),
                    stop=(j == DCH - 1),
                )
            nc.vector.tensor_copy(vt[:, t16, :, 0:DK], psv[:])

    # ---- Phases C+D: attention + out_proj, per 512-wide t1 chunk ----
    from collections import deque

    pt_pool = ctx.enter_context(tc.tile_pool(name="pt", bufs=3))
    lrp = ctx.enter_context(tc.tile_pool(name="lrp", bufs=2))
    outp = ctx.enter_context(tc.tile_pool(name="outp", bufs=2))
    ps_s_pool = ctx.enter_context(tc.tile_pool(name="ps_s", bufs=2, space="PSUM"))
    ps_av_pool = ctx.enter_context(tc.tile_pool(name="ps_av", bufs=2, space="PSUM"))
    ps_o_pool = ctx.enter_context(tc.tile_pool(name="ps_o", bufs=1, space="PSUM"))
    ps_b_pool = ctx.enter_context(tc.tile_pool(name="ps_b", bufs=1, space="PSUM"))

    av_tiles = {}
    cur_l = [None]  # [NH, 512] tile gathering the chunk's softmax denominators
    pend_back = deque()  # per-head normalize back-halves awaiting emission

    def emit_outproj(c):
        for ti in range(4):
            t0 = c * 512 + ti * P
            for eo in range(2):
                pso = ps_o_pool.tile([P, 512], FP32, name="ps_o")
                for s in range(EC):
                    nc.tensor.matmul(
                        pso[:],
                        lhsT=attn[:, s, t0 : t0 + P],
                        rhs=wo_sb[:, s, eo * 512 : (eo + 1) * 512],
                        start=(s == 0),
                        stop=(s == EC - 1),
                    )
                ob = outp.tile([P, 512], FP32, name="ob")
                nc.vector.tensor_copy(ob[:], pso[:])
                nc.sync.dma_start(out[t0 : t0 + P, eo * 512 : (eo + 1) * 512], ob[:])

    def emit_back(c, h, rec):
        # attn[dk, t1] *= 1/l_h (in place); PE K=1 bcast + DVE multiply
        po = (h % 2) * DK
        sub = h // 2
        sl = attn[po : po + DK, sub, c * 512 : (c + 1) * 512]
        psb = ps_b_pool.tile([DK, 512], FP32, name="ps_b")
        nc.tensor.matmul(
            psb[:],
            lhsT=sel_sb[:, h * DK : (h + 1) * DK],
            rhs=rec[:],
            start=True,
            stop=True,
        )
        nc.vector.tensor_tensor(sl, sl, psb[:], mybir.AluOpType.mult)
        if h == NH - 1:
            emit_outproj(c)

    def finish_head(c, h, ps_av):
        po = (h % 2) * DK
        sub = h // 2
        if h == 0:
            cur_l[0] = lrp.tile([NH, 512], FP32, name="lall")
            nc.vector.memset(cur_l[0][:], 0.0)
        # evacuate psum: unnormalized AV -> attn (bf16); denominator rows
        # (head h on partition DK+h, zeros elsewhere) accumulate into lall
        nc.vector.tensor_copy(
            attn[po : po + DK, sub, c * 512 : (c + 1) * 512], ps_av[0:DK, :]
        )
        nc.vector.tensor_tensor(
            cur_l[0][:], cur_l[0][:], ps_av[DK : DK + NH, :], mybir.AluOpType.add
        )
        if h == NH - 1:
            rec = lrp.tile([NH, 512], FP32R, name="rec")
            nc.vector.reciprocal(rec[:], cur_l[0][:])  # one batched recip/chunk
            for hh in range(NH):
                pend_back.append((c, hh, rec))

    def emit_front(c, h, jp, njt):
        po = (h % 2) * DK
        sub = h // 2
        pair = (2 * jp, 2 * jp + 1)
        ps2 = ps_s_pool.tile([P, 1024], FP32, name="ps_s")
        pt2 = pt_pool.tile([P, 1024], BF16, name="pt")
        for sl, j in enumerate(pair):
            m = j - 4 * c
            off = 0 if m < 0 else m * P
            t1lo = c * 512 + off
            base = sl * 512
            nc.tensor.matmul(
                ps2[:, base : base + 512 - off],
                lhsT=kT[po : po + DK, sub, j * P : (j + 1) * P],
                rhs=qT[po : po + DK, sub, t1lo : (c + 1) * 512],
                start=True,
                stop=True,
            )
            if m >= 0:
                nc.vector.tensor_tensor(
                    ps2[:, base : base + P],
                    ps2[:, base : base + P],
                    mtri_sb[:],
                    mybir.AluOpType.add,
                )
        if pair[1] < 4 * c:  # both tiles full-width: one fused exp
            nc.scalar.activation(
                pt2[:], ps2[:], mybir.ActivationFunctionType.Exp, scale=0.125
            )
        else:  # diagonal: exact widths only (gap cols are unwritten PSUM)
            for sl, j in enumerate(pair):
                off = max(0, j - 4 * c) * P
                base = sl * 512
                nc.scalar.activation(
                    pt2[:, base : base + 512 - off],
                    ps2[:, base : base + 512 - off],
                    mybir.ActivationFunctionType.Exp,
                    scale=0.125,
                )
        return pt2

    def emit_av(c, h, jp, njt, pt2):
        if jp == 0:
            av_tiles[(c, h)] = ps_av_pool.tile([DK + NH, 512], FP32, name="ps_av")
        ps_av = av_tiles[(c, h)]
        for sl, j in enumerate((2 * jp, 2 * jp + 1)):
            m = j - 4 * c
            off = 0 if m < 0 else m * P
            base = sl * 512
            nc.tensor.matmul(
                ps_av[:, off:512],
                lhsT=vt[:, j, h, :],
                rhs=pt2[:, base : base + 512 - off],
                start=(j == 0),
                stop=(j == njt - 1),
                skip_group_check=True,
            )
        if jp == njt // 2 - 1:
            finish_head(c, h, ps_av)

    prev = None  # AV emission lags the S/exp stream by one pair
    for c in range(NT512):
        for h in range(NH):
            njt = 4 * c + 4
            for jp in range(njt // 2):
                pt2 = emit_front(c, h, jp, njt)
                if prev is not None:
                    emit_av(*prev)
                prev = (c, h, jp, njt, pt2)
                for _ in range(2):  # drain normalize back-halves, spread out
                    if pend_back:
                        emit_back(*pend_back.popleft())
    emit_av(*prev)
    while pend_back:
        emit_back(*pend_back.popleft())


def _build_nc():
    from contextlib import ExitStack

    nc = bacc.Bacc("TRN2", target_bir_lowering=False, debug=False)
    xT = nc.dram_tensor("xT", [D, T], BF16, kind="ExternalInput")
    wq = nc.dram_tensor("wq", [D, E], BF16, kind="ExternalInput")
    wk = nc.dram_tensor("wk", [D, E], BF16, kind="ExternalInput")
    wv = nc.dram_tensor("wv", [D, E], BF16, kind="ExternalInput")
    wo = nc.dram_tensor("wo", [E, D], BF16, kind="ExternalInput")
    mtri = nc.dram_tensor("mtri", [P, P], FP32, kind="ExternalInput")
    sel = nc.dram_tensor("sel", [NH, NH * DK], FP32R, kind="ExternalInput")
    out = nc.dram_tensor("out", [T, D], FP32, kind="ExternalOutput")

    with (
        tile.TileContext(nc) as tc,
        nc.allow_low_precision(reason="f32r/bf16 intermediates by design"),
        ExitStack() as ctx,
    ):
        _emit(
            nc, tc, ctx, xT[:], wq[:], wk[:], wv[:], wo[:], mtri[:], sel[:], out[:]
        )
    nc.compile()
    return nc


def _host_inputs(x, W_qkv, W_o):
    x = np.asarray(x, dtype=np.float32)
    W_qkv = np.asarray(W_qkv, dtype=np.float32)
    W_o = np.asarray(W_o, dtype=np.float32)
    mtri = np.tril(np.full((P, P), MASK_VAL, dtype=np.float32), -1)
    sel = np.zeros((NH, NH * DK), dtype=np.float32)
    for hh in range(NH):
        sel[hh, hh * DK : (hh + 1) * DK] = 1.0
    bf = ml_dtypes.bfloat16
    in_maps = []
    for b in range(B):
        xTb = np.ascontiguousarray(x[b].T.astype(bf))
        for hg in range(2):
            sl = slice(E * hg, E * hg + E)
            in_maps.append(
                {
                    "xT": xTb,
                    "wq": np.ascontiguousarray(W_qkv[0 * D :][sl].T.astype(bf)),
                    "wk": np.ascontiguousarray(W_qkv[1 * D :][sl].T.astype(bf)),
                    "wv": np.ascontiguousarray(W_qkv[2 * D :][sl].T.astype(bf)),
                    "wo": np.ascontiguousarray(
                        W_o[:, sl].T.astype(ml_dtypes.bfloat16)
                    ),
                    "mtri": mtri,
                    "sel": sel,
                }
            )
    return in_maps


def _run(x, W_qkv, W_o, trace=False, tmpdir=None):
    nc = _build_nc()
    in_maps = _host_inputs(x, W_qkv, W_o)
    res = run_bass_kernel_spmd(
        nc, in_maps, list(range(NCORES)), trace=trace, tmpdir=tmpdir
    )
    out = np.empty((B, T, D), dtype=np.float32)
    for b in range(B):
        out[b] = res.results[2 * b]["out"] + res.results[2 * b + 1]["out"]
    return out, res.exec_time_ns


def kernel(x, W_qkv, W_o):
    out, _ = _run(x, W_qkv, W_o, trace=False)
    return out



# revision 3
# speedup vs baseline: 1.3848x; 1.3848x over previous
"""Causal multi-head self-attention on 8 TRN2 NeuronCores.

Problem (hardcoded): x (4, 2048, 1024) f32, W_qkv (3072, 1024), W_o (1024, 1024).
  qkv = x @ W_qkv.T; q,k,v split -> (B,H,T,DK) with H=16, DK=64
  scores = q k^T / 8 + causal mask; attn = softmax; out = (attn v) @ W_o.T

Sharding: core = 2*b + hg  (b in 0..3 batches, hg in 0..1 head-groups of 8 heads).
Each core computes a partial out[b] over its 512 attn columns; host sums pairs.

Per-core dataflow ("T-attention": t2-on-partitions S^T tiles), single fused
stream built to keep the PE continuously busy (TRN2 PE p-state drops 2.4GHz
-> 1.2GHz on any stall, so stalls cost double):
  - S^T tile [t2:128, t1<=512] = kT_h^T-slice x qT_h-slice (K=DK=64, bf16);
    causal handled by skipping fully-masked column ranges and one [128,128]
    triangle mask add on the diagonal block.
  - exp via ACT with fused 1/8 scale (no max-subtraction; |scores|/8 <~ 2).
  - AV: psum[72, 512] += v_aug^T-slice x P^T tile (rows 64..71 = row sums l).
    AV pairs are emitted TWO pairs behind the S/exp stream (lag-2) so the PE
    never waits on the ACT exp.
  - All projection work (qkv projections for the NEXT chunk, out_proj of
    earlier chunks, normalize broadcasts) is interleaved into the S/AV slot
    stream as dependency-free PE filler, so the PE queue never runs dry.
  - normalize: batched reciprocal_approx_fast of l per chunk, partition-
    broadcast via a K=8 fp16 PE matmul, multiply on DVE.
  - out_proj: attn^T x W_o-slice, f32 partial to DRAM.
"""

import sys
from collections import deque

import numpy as np

sys.path.insert(0, "/opt/trn_rl_repo")

import ml_dtypes  # noqa: E402

from concourse import bacc, bass, mybir, tile  # noqa: E402
from concourse.bass_utils import run_bass_kernel_spmd  # noqa: E402

FP32 = mybir.dt.float32
FP16 = mybir.dt.float16
BF16 = mybir.dt.bfloat16

B, T, D, H, DK = 4, 2048, 1024, 16, 64
NCORES = 8
E = 512          # qkv columns per head-group
NH = 8           # heads per core
P = 128
DCH = D // P     # 8 contraction chunks for the projections
EC = E // P      # 4 e-chunks for q/k
NT512 = T // 512
NT128 = T // P

MASK_VAL = -1e9
AV_LAG = 2                     # AV pairs trail the S/exp stream by this many
DRAIN_BUDGET = [6, 4, 3, 2]    # filler matmuls per slot, per chunk


def _emit(nc, tc, ctx, xT, wq, wk, wv, wo, mtri, sel, out):
    consts = ctx.enter_context(tc.tile_pool(name="consts", bufs=1))
    persist = ctx.enter_context(tc.tile_pool(name="persist", bufs=1))

    # Persistent SBUF state
    qT = persist.tile([P, EC, T], BF16, name="qT")        # e = ec*128+p
    kT = persist.tile([P, EC, T], BF16, name="kT")
    vt = persist.tile([P, NT128, NH, DK + NH], BF16, name="vt")  # + one-hot cols
    attn = persist.tile([P, EC, T], BF16, name="attn")    # dl = s*128+p
    wo_sb = persist.tile([P, EC, D], BF16, name="wo_sb")  # dl = s*128+p
    xT_sb = persist.tile([P, DCH, T], BF16, name="xT_sb")
    wq_sb = persist.tile([P, DCH, E], BF16, name="wq_sb")
    wk_sb = persist.tile([P, DCH, E], BF16, name="wk_sb")
    wv_sb = persist.tile([P, DCH, E], BF16, name="wv_sb")
    mtri_sb = consts.tile([P, P], FP32, name="mtri_sb")
    sel_sb = consts.tile([NH, NH * DK], FP16, name="sel_sb")

    nc.sync.dma_start(mtri_sb[:], mtri[:])
    nc.sync.dma_start(sel_sb[:], sel[:])
    # k first (kproj runs first), then v, q; wo last (first needed much later)
    for j in range(DCH):
        nc.sync.dma_start(xT_sb[:, j], xT[j * P : (j + 1) * P, :])
        nc.sync.dma_start(wk_sb[:, j], wk[j * P : (j + 1) * P, :])
    for j in range(DCH):
        nc.sync.dma_start(wv_sb[:, j], wv[j * P : (j + 1) * P, :])
    for j in range(DCH):
        nc.sync.dma_start(wq_sb[:, j], wq[j * P : (j + 1) * P, :])
    for s in range(EC):
        nc.sync.dma_start(wo_sb[:, s], wo[s * P : (s + 1) * P, :])

    # head h's ones column sits at DK+h so its denominators land on a
    # distinct psum partition (64+h); other heads' columns there are zero
    nc.vector.memset(vt[:, :, :, DK:], 0.0)
    for hh in range(NH):
        nc.vector.memset(vt[:, :, hh, DK + hh], 1.0)

    pt_pool = ctx.enter_context(tc.tile_pool(name="pt", bufs=AV_LAG + 2))
    lrp = ctx.enter_context(tc.tile_pool(name="lrp", bufs=6))
    obuf = ctx.enter_context(tc.tile_pool(name="obuf", bufs=2))
    ps_s_pool = ctx.enter_context(tc.tile_pool(name="ps_s", bufs=2, space="PSUM"))
    ps_av_pool = ctx.enter_context(tc.tile_pool(name="ps_av", bufs=2, space="PSUM"))
    # shared ring for qkv-proj / out_proj / normalize-broadcast psums
    ps_misc = ctx.enter_context(tc.tile_pool(name="ps_misc", bufs=2, space="PSUM"))

    # ---- filler machinery: dep-free PE work interleaved into the stream ----
    filler_q = deque()   # generator objects, FIFO; ~1 matmul per step
    steps_left = [0]

    def push_filler(gen, nsteps):
        filler_q.append(gen)
        steps_left[0] += nsteps

    def drain(n):
        for _ in range(n):
            while filler_q:
                try:
                    next(filler_q[0])
                    steps_left[0] -= 1
                    break
                except StopIteration:
                    filler_q.popleft()
            else:
                return

    def gen_qkproj(w_sb, dst, c):
        for ec in range(EC):
            ps = ps_misc.tile([P, 512], FP32, name="ps_p")
            for j in range(DCH):
                nc.tensor.matmul(
                    ps[:],
                    lhsT=w_sb[:, j, ec * P : (ec + 1) * P],
                    rhs=xT_sb[:, j, c * 512 : (c + 1) * 512],
                    start=(j == 0),
                    stop=(j == DCH - 1),
                    skip_group_check=True,
                )
                if j < DCH - 1:
                    yield
            nc.vector.tensor_copy(dst[:, ec, c * 512 : (c + 1) * 512], ps[:])
            yield

    def gen_vproj(c):
        for t16 in range(4 * c, 4 * c + 4):
            psv = ps_misc.tile([P, NH, DK], FP32, name="ps_p")
            for j in range(DCH):
                nc.tensor.matmul(
                    psv[:],
                    lhsT=xT_sb[:, j, t16 * P : (t16 + 1) * P],
                    rhs=wv_sb[:, j, :],
                    start=(j == 0),
                    stop=(j == DCH - 1),
                    skip_group_check=True,
                )
                if j < DCH - 1:
                    yield
            nc.vector.tensor_copy(vt[:, t16, :, 0:DK], psv[:])
            yield

    def gen_outproj(c):
        for ti in range(4):
            t0 = c * 512 + ti * P
            for eo in range(2):
                pso = ps_misc.tile([P, 512], FP32, name="ps_p")
                for s in range(EC):
                    nc.tensor.matmul(
                        pso[:],
                        lhsT=attn[:, s, t0 : t0 + P],
                        rhs=wo_sb[:, s, eo * 512 : (eo + 1) * 512],
                        start=(s == 0),
                        stop=(s == EC - 1),
                        skip_group_check=True,
                    )
                    if s < EC - 1:
                        yield
                ob = obuf.tile([P, 512], FP32, name="ob")
                nc.vector.tensor_copy(ob[:], pso[:])
                nc.sync.dma_start(out[t0 : t0 + P, eo * 512 : (eo + 1) * 512], ob[:])
                yield

    def gen_backs(c, rec16):
        # attn[dk, t1] *= 1/l_h (in place); PE K=8 fp16 bcast + DVE multiply
        for h in range(NH):
            po = (h % 2) * DK
            sub = h // 2
            sl = attn[po : po + DK, sub, c * 512 : (c + 1) * 512]
            psb = ps_misc.tile([DK, 512], FP32, name="ps_p")
            nc.tensor.matmul(
                psb[:],
                lhsT=sel_sb[:, h * DK : (h + 1) * DK],
                rhs=rec16[:],
                start=True,
                stop=True,
                skip_group_check=True,
            )
            nc.vector.tensor_tensor(sl, sl, psb[:], mybir.AluOpType.mult)
            yield

    # ---- attention stream ----
    av_tiles = {}
    cur_l = [None]  # [NH, 512] tile gathering the chunk's softmax denominators

    def finish_head(c, h, ps_av):
        po = (h % 2) * DK
        sub = h // 2
        # evacuate psum: unnormalized AV -> attn (bf16); denominator rows
        # (head h on partition DK+h, zeros elsewhere) accumulate into lall
        nc.vector.tensor_copy(
            attn[po : po + DK, sub, c * 512 : (c + 1) * 512], ps_av[0:DK, :]
        )
        if h == 0:
            cur_l[0] = lrp.tile([NH, 512], FP32, name="lall")
            nc.vector.tensor_copy(cur_l[0][:], ps_av[DK : DK + NH, :])
        else:
            nc.vector.tensor_tensor(
                cur_l[0][:], cur_l[0][:], ps_av[DK : DK + NH, :], mybir.AluOpType.add
            )
        if h == NH - 1:
            rec32 = lrp.tile([NH, 512], FP32, name="rec32")
            nc.vector.reciprocal_approx_fast(out=rec32[:], in_=cur_l[0][:])
            rec16 = lrp.tile([NH, 512], FP16, name="rec16")
            nc.vector.tensor_copy(rec16[:], rec32[:])
            push_filler(gen_backs(c, rec16[:]), NH)
            if c >= 2:  # earlier chunks' out_proj is scheduled at chunk starts
                push_filler(gen_outproj(c), 32)

    def emit_front(c, h, jp, njt):
        po = (h % 2) * DK
        sub = h // 2
        pair = (2 * jp, 2 * jp + 1)
        ps2 = ps_s_pool.tile([P, 1024], FP32, name="ps_s")
        pt2 = pt_pool.tile([P, 1024], BF16, name="pt")
        for sl, j in enumerate(pair):
            m = j - 4 * c
            off = 0 if m < 0 else m * P
            t1lo = c * 512 + off
            base = sl * 512
            nc.tensor.matmul(
                ps2[:, base : base + 512 - off],
                lhsT=kT[po : po + DK, sub, j * P : (j + 1) * P],
                rhs=qT[po : po + DK, sub, t1lo : (c + 1) * 512],
                start=True,
                stop=True,
                skip_group_check=True,
            )
            if m >= 0:
                nc.vector.tensor_tensor(
                    ps2[:, base : base + P],
                    ps2[:, base : base + P],
                    mtri_sb[:],
                    mybir.AluOpType.add,
                )
        if pair[1] < 4 * c:  # both tiles full-width: one fused exp
            nc.scalar.activation(
                pt2[:], ps2[:], mybir.ActivationFunctionType.Exp, scale=0.125
            )
        else:  # diagonal: exact widths only (gap cols are unwritten PSUM)
            for sl, j in enumerate(pair):
                off = max(0, j - 4 * c) * P
                base = sl * 512
                nc.scalar.activation(
                    pt2[:, base : base + 512 - off],
                    ps2[:, base : base + 512 - off],
                    mybir.ActivationFunctionType.Exp,
                    scale=0.125,
                )
        return pt2

    def emit_av(c, h, jp, njt, pt2):
        if jp == 0:
            av_tiles[(c, h)] = ps_av_pool.tile([DK + NH, 512], FP32, name="ps_av")
        ps_av = av_tiles[(c, h)]
        for sl, j in enumerate((2 * jp, 2 * jp + 1)):
            m = j - 4 * c
            off = 0 if m < 0 else m * P
            base = sl * 512
            nc.tensor.matmul(
                ps_av[:, off:512],
                lhsT=vt[:, j, h, :],
                rhs=pt2[:, base : base + 512 - off],
                start=(j == 0),
                stop=(j == njt - 1),
                skip_group_check=True,
            )
        if jp == njt // 2 - 1:
            finish_head(c, h, av_tiles.pop((c, h)))

    # ---- prologue: qkv projections for chunk 0, emitted densely ----
    for g in (
        gen_qkproj(wk_sb, kT, 0),
        gen_vproj(0),
        gen_qkproj(wq_sb, qT, 0),
    ):
        for _ in g:
            pass

    # ---- main loop: S/exp/AV slots with interleaved filler ----
    av_q = deque()
    drained_total = [0]
    deadline = [0]  # fillers that must drain before the next chunk starts

    def drain_tracked(n):
        before = steps_left[0]
        drain(n)
        drained_total[0] += before - steps_left[0]

    for c in range(NT512):
        if c < NT512 - 1:
            push_filler(gen_qkproj(wk_sb, kT, c + 1), 4 * DCH)
            push_filler(gen_vproj(c + 1), 4 * DCH)
            push_filler(gen_qkproj(wq_sb, qT, c + 1), 4 * DCH)
            deadline[0] = drained_total[0] + steps_left[0]
        if c == 2:
            push_filler(gen_outproj(0), 32)
        if c == 3:
            push_filler(gen_outproj(1), 32)
        njt = 4 * (c + 1)
        for h in range(NH):
            for jp in range(njt // 2):
                pt2 = emit_front(c, h, jp, njt)
                av_q.append((c, h, jp, njt, pt2))
                if len(av_q) > AV_LAG:
                    emit_av(*av_q.popleft())
                drain_tracked(DRAIN_BUDGET[c])
        # make sure next chunk's projections finished draining
        while drained_total[0] < deadline[0]:
            drain_tracked(4)

    while av_q:
        emit_av(*av_q.popleft())
    while filler_q:
        drain_tracked(8)


def _build_nc():
    from contextlib import ExitStack

    nc = bacc.Bacc("TRN2", target_bir_lowering=False, debug=False)
    xT = nc.dram_tensor("xT", [D, T], BF16, kind="ExternalInput")
    wq = nc.dram_tensor("wq", [D, E], BF16, kind="ExternalInput")
    wk = nc.dram_tensor("wk", [D, E], BF16, kind="ExternalInput")
    wv = nc.dram_tensor("wv", [D, E], BF16, kind="ExternalInput")
    wo = nc.dram_tensor("wo", [E, D], BF16, kind="ExternalInput")
    mtri = nc.dram_tensor("mtri", [P, P], FP32, kind="ExternalInput")
    sel = nc.dram_tensor("sel", [NH, NH * DK], FP16, kind="ExternalInput")
    out = nc.dram_tensor("out", [T, D], FP32, kind="ExternalOutput")

    with (
        tile.TileContext(nc) as tc,
        nc.allow_low_precision(reason="f16/bf16 intermediates by design"),
        ExitStack() as ctx,
    ):
        _emit(
            nc, tc, ctx, xT[:], wq[:], wk[:], wv[:], wo[:], mtri[:], sel[:], out[:]
        )
    nc.compile()
    return nc


def _host_inputs(x, W_qkv, W_o):
    x = np.asarray(x, dtype=np.float32)
    W_qkv = np.asarray(W_qkv, dtype=np.float32)
    W_o = np.asarray(W_o, dtype=np.float32)
    mtri = np.tril(np.full((P, P), MASK_VAL, dtype=np.float32), -1)
    sel = np.zeros((NH, NH * DK), dtype=np.float16)
    for hh in range(NH):
        sel[hh, hh * DK : (hh + 1) * DK] = 1.0
    bf = ml_dtypes.bfloat16
    in_maps = []
    for b in range(B):
        xTb = np.ascontiguousarray(x[b].T.astype(bf))
        for hg in range(2):
            sl = slice(E * hg, E * hg + E)
            in_maps.append(
                {
                    "xT": xTb,
                    "wq": np.ascontiguousarray(W_qkv[0 * D :][sl].T.astype(bf)),
                    "wk": np.ascontiguousarray(W_qkv[1 * D :][sl].T.astype(bf)),
                    "wv": np.ascontiguousarray(W_qkv[2 * D :][sl].T.astype(bf)),
                    "wo": np.ascontiguousarray(
                        W_o[:, sl].T.astype(ml_dtypes.bfloat16)
                    ),
                    "mtri": mtri,
                    "sel": sel,
                }
            )
    return in_maps


def _run(x, W_qkv, W_o, trace=False, tmpdir=None):
    nc = _build_nc()
    in_maps = _host_inputs(x, W_qkv, W_o)
    res = run_bass_kernel_spmd(
        nc, in_maps, list(range(NCORES)), trace=trace, tmpdir=tmpdir
    )
    out = np.empty((B, T, D), dtype=np.float32)
    for b in range(B):
        out[b] = res.results[2 * b]["out"] + res.results[2 * b + 1]["out"]
    return out, res.exec_time_ns


def kernel(x, W_qkv, W_o):
    out, _ = _run(x, W_qkv, W_o, trace=False)
    return out


# revision 16
# speedup vs baseline: 1.4280x; 1.0312x over previous
"""Causal multi-head self-attention on 8 TRN2 NeuronCores.

Problem (hardcoded): x (4, 2048, 1024) f32, W_qkv (3072, 1024), W_o (1024, 1024).
  qkv = x @ W_qkv.T; q,k,v split -> (B,H,T,DK) with H=16, DK=64
  scores = q k^T / 8 + causal mask; attn = softmax; out = (attn v) @ W_o.T

Sharding: core = 2*b + hg  (b in 0..3 batches, hg in 0..1 head-groups of 8 heads).
Each core computes a partial out[b] over its 512 attn columns; host sums pairs.

Per-core dataflow ("T-attention": t2-on-partitions S^T tiles), single fused
stream built to keep the PE continuously busy (TRN2 PE p-state drops 2.4GHz
-> 1.2GHz on any stall, so stalls cost double):
  - S^T tile [t2:128, t1<=512] = kT_h^T-slice x qT_h-slice (K=DK=64, bf16);
    causal handled by skipping fully-masked column ranges and one [128,128]
    triangle mask add on the diagonal block.
  - exp via ACT with fused 1/8 scale (no max-subtraction; |scores|/8 <~ 2).
  - AV: psum[72, 512] += v_aug^T-slice x P^T tile (rows 64..71 = row sums l).
    AV pairs are emitted TWO pairs behind the S/exp stream (lag-2) so the PE
    never waits on the ACT exp.
  - All projection work (qkv projections for the NEXT chunk, out_proj of
    earlier chunks, normalize broadcasts) is interleaved into the S/AV slot
    stream as dependency-free PE filler, so the PE queue never runs dry.
  - normalize: batched reciprocal_approx_fast of l per chunk, partition-
    broadcast via a K=8 fp16 PE matmul, multiply on DVE.
  - out_proj: attn^T x W_o-slice, f32 partial to DRAM.
"""

import sys
from collections import deque

import numpy as np

sys.path.insert(0, "/opt/trn_rl_repo")

import ml_dtypes  # noqa: E402

from concourse import bacc, bass, mybir, tile  # noqa: E402
from concourse.bass_utils import run_bass_kernel_spmd  # noqa: E402

FP32 = mybir.dt.float32
FP16 = mybir.dt.float16
BF16 = mybir.dt.bfloat16

B, T, D, H, DK = 4, 2048, 1024, 16, 64
NCORES = 8
E = 512          # qkv columns per head-group
NH = 8           # heads per core
P = 128
DCH = D // P     # 8 contraction chunks for the projections
EC = E // P      # 4 e-chunks for q/k
NT512 = T // 512
NT128 = T // P

MASK_VAL = -1e9
AV_LAG = 2                     # AV pairs trail the S/exp stream by this many
DRAIN_BUDGET = [6, 4, 3, 2]    # filler matmuls per slot, per chunk


def _emit(nc, tc, ctx, xT, wq, wk, wv, wo, mtri, sel, out):
    consts = ctx.enter_context(tc.tile_pool(name="consts", bufs=1))
    persist = ctx.enter_context(tc.tile_pool(name="persist", bufs=1))

    # Persistent SBUF state
    qT = persist.tile([P, EC, T], BF16, name="qT")        # e = ec*128+p
    kT = persist.tile([P, EC, T], BF16, name="kT")
    vt = persist.tile([P, NT128, NH, DK + NH], BF16, name="vt")  # + one-hot cols
    attn = persist.tile([P, EC, T], BF16, name="attn")    # dl = s*128+p
    wo_sb = persist.tile([P, EC, D], BF16, name="wo_sb")  # dl = s*128+p
    xT_sb = persist.tile([P, DCH, T], BF16, name="xT_sb")
    wq_sb = persist.tile([P, DCH, E], BF16, name="wq_sb")
    wk_sb = persist.tile([P, DCH, E], BF16, name="wk_sb")
    wv_sb = persist.tile([P, DCH, E], BF16, name="wv_sb")
    mtri_sb = consts.tile([P, P], FP32, name="mtri_sb")
    sel_sb = consts.tile([NH, NH * DK], FP16, name="sel_sb")

    nc.sync.dma_start(mtri_sb[:], mtri[:])
    nc.sync.dma_start(sel_sb[:], sel[:])
    # k first (kproj runs first), then v, q; wo last (first needed much later)
    for j in range(DCH):
        nc.sync.dma_start(xT_sb[:, j], xT[j * P : (j + 1) * P, :])
        nc.sync.dma_start(wk_sb[:, j], wk[j * P : (j + 1) * P, :])
    for j in range(DCH):
        nc.sync.dma_start(wv_sb[:, j], wv[j * P : (j + 1) * P, :])
    for j in range(DCH):
        nc.sync.dma_start(wq_sb[:, j], wq[j * P : (j + 1) * P, :])
    for s in range(EC):
        nc.sync.dma_start(wo_sb[:, s], wo[s * P : (s + 1) * P, :])

    # head h's ones column sits at DK+h so its denominators land on a
    # distinct psum partition (64+h); other heads' columns there are zero
    nc.vector.memset(vt[:, :, :, DK:], 0.0)
    for hh in range(NH):
        nc.vector.memset(vt[:, :, hh, DK + hh], 1.0)

    pt_pool = ctx.enter_context(tc.tile_pool(name="pt", bufs=AV_LAG + 2))
    lrp = ctx.enter_context(tc.tile_pool(name="lrp", bufs=8))
    obuf = ctx.enter_context(tc.tile_pool(name="obuf", bufs=2))
    ps_s_pool = ctx.enter_context(tc.tile_pool(name="ps_s", bufs=2, space="PSUM"))
    ps_av_pool = ctx.enter_context(tc.tile_pool(name="ps_av", bufs=2, space="PSUM"))
    # shared ring for qkv-proj / out_proj / normalize-broadcast psums
    ps_misc = ctx.enter_context(tc.tile_pool(name="ps_misc", bufs=2, space="PSUM"))

    # PE p-state warmup: ~10 junk matmuls ramp the clock while input DMAs land
    warm = consts.tile([P, 512], BF16, name="warm")
    nc.vector.memset(warm[:], 0.0)
    for _ in range(10):
        pw = ps_misc.tile([P, 512], FP32, name="ps_p")
        nc.tensor.matmul(
            pw[:], lhsT=warm[:, 0:P], rhs=warm[:],
            start=True, stop=True, skip_group_check=True,
        )

    # ---- filler machinery: dep-free PE work interleaved into the stream ----
    filler_q = deque()   # generator objects, FIFO; ~1 matmul per step
    steps_left = [0]

    def push_filler(gen, nsteps):
        filler_q.append(gen)
        steps_left[0] += nsteps

    def drain(n):
        for _ in range(n):
            while filler_q:
                try:
                    next(filler_q[0])
                    steps_left[0] -= 1
                    break
                except StopIteration:
                    filler_q.popleft()
            else:
                return

    def gen_qkproj(w_sb, dst, c):
        for ec in range(EC):
            ps = ps_misc.tile([P, 512], FP32, name="ps_p")
            for j in range(DCH):
                nc.tensor.matmul(
                    ps[:],
                    lhsT=w_sb[:, j, ec * P : (ec + 1) * P],
                    rhs=xT_sb[:, j, c * 512 : (c + 1) * 512],
                    start=(j == 0),
                    stop=(j == DCH - 1),
                    skip_group_check=True,
                )
                if j < DCH - 1:
                    yield
            nc.vector.tensor_copy(dst[:, ec, c * 512 : (c + 1) * 512], ps[:])
            yield

    def gen_vproj(c):
        for t16 in range(4 * c, 4 * c + 4):
            psv = ps_misc.tile([P, NH, DK], FP32, name="ps_p")
            for j in range(DCH):
                nc.tensor.matmul(
                    psv[:],
                    lhsT=xT_sb[:, j, t16 * P : (t16 + 1) * P],
                    rhs=wv_sb[:, j, :],
                    start=(j == 0),
                    stop=(j == DCH - 1),
                    skip_group_check=True,
                )
                if j < DCH - 1:
                    yield
            nc.vector.tensor_copy(vt[:, t16, :, 0:DK], psv[:])
            yield

    def gen_outproj(c):
        for ti in range(4):
            t0 = c * 512 + ti * P
            for eo in range(2):
                pso = ps_misc.tile([P, 512], FP32, name="ps_p")
                for s in range(EC):
                    nc.tensor.matmul(
                        pso[:],
                        lhsT=attn[:, s, t0 : t0 + P],
                        rhs=wo_sb[:, s, eo * 512 : (eo + 1) * 512],
                        start=(s == 0),
                        stop=(s == EC - 1),
                        skip_group_check=True,
                    )
                    if s < EC - 1:
                        yield
                ob = obuf.tile([P, 512], FP32, name="ob")
                nc.vector.tensor_copy(ob[:], pso[:])
                nc.sync.dma_start(out[t0 : t0 + P, eo * 512 : (eo + 1) * 512], ob[:])
                yield

    def gen_back_pair(c, s, rec16):
        # attn[:, s, t1] *= 1/l_{2s,2s+1} (in place, both heads of the pair);
        # K=8 fp16 PE partition-broadcast + one [128,512] DVE multiply
        sl = attn[:, s, c * 512 : (c + 1) * 512]
        psb = ps_misc.tile([P, 512], FP32, name="ps_p")
        nc.tensor.matmul(
            psb[:],
            lhsT=sel_sb[:, s * P : (s + 1) * P],
            rhs=rec16,
            start=True,
            stop=True,
            skip_group_check=True,
        )
        nc.vector.tensor_tensor(sl, sl, psb[:], mybir.AluOpType.mult)
        yield

    # ---- attention stream ----
    av_tiles = {}
    cur_l = [None]  # [NH, 512] tile gathering the chunk's softmax denominators

    def finish_head(c, h, ps_av):
        po = (h % 2) * DK
        sub = h // 2
        # evacuate psum: unnormalized AV -> attn (bf16); denominator rows
        # (head h on partition DK+h, zeros elsewhere) accumulate into lall
        nc.vector.tensor_copy(
            attn[po : po + DK, sub, c * 512 : (c + 1) * 512], ps_av[0:DK, :]
        )
        if h == 0:
            cur_l[0] = lrp.tile([NH, 512], FP32, name="lall")
            nc.vector.tensor_copy(cur_l[0][:], ps_av[DK : DK + NH, :])
        else:
            nc.vector.tensor_tensor(
                cur_l[0][:], cur_l[0][:], ps_av[DK : DK + NH, :], mybir.AluOpType.add
            )
        if h == NH - 1:
            rec32 = lrp.tile([NH, 512], FP32, name="rec32")
            nc.vector.reciprocal_approx_fast(out=rec32[:], in_=cur_l[0][:])
            rec16 = lrp.tile([NH, 512], FP16, name="rec16")
            nc.vector.tensor_copy(rec16[:], rec32[:])
            for s in range(EC):
                push_filler(gen_back_pair(c, s, rec16[:]), 1)
            if c >= 2:
                # earlier chunks' out_proj is scheduled at chunk starts
                push_filler(gen_outproj(c), 32)

    def emit_front(c, h, jp, njt):
        po = (h % 2) * DK
        sub = h // 2
        pair = (2 * jp, 2 * jp + 1)
        ps2 = ps_s_pool.tile([P, 1024], FP32, name="ps_s")
        pt2 = pt_pool.tile([P, 1024], BF16, name="pt")
        for sl, j in enumerate(pair):
            m = j - 4 * c
            off = 0 if m < 0 else m * P
            t1lo = c * 512 + off
            base = sl * 512
            nc.tensor.matmul(
                ps2[:, base : base + 512 - off],
                lhsT=kT[po : po + DK, sub, j * P : (j + 1) * P],
                rhs=qT[po : po + DK, sub, t1lo : (c + 1) * 512],
                start=True,
                stop=True,
                skip_group_check=True,
            )
            if m >= 0:
                nc.vector.tensor_tensor(
                    ps2[:, base : base + P],
                    ps2[:, base : base + P],
                    mtri_sb[:],
                    mybir.AluOpType.add,
                )
        if pair[1] < 4 * c:  # both tiles full-width: one fused exp
            nc.scalar.activation(
                pt2[:], ps2[:], mybir.ActivationFunctionType.Exp, scale=0.125
            )
        else:  # diagonal: exact widths only (gap cols are unwritten PSUM)
            for sl, j in enumerate(pair):
                off = max(0, j - 4 * c) * P
                base = sl * 512
                nc.scalar.activation(
                    pt2[:, base : base + 512 - off],
                    ps2[:, base : base + 512 - off],
                    mybir.ActivationFunctionType.Exp,
                    scale=0.125,
                )
        return pt2

    def emit_av(c, h, jp, njt, pt2):
        if jp == 0:
            av_tiles[(c, h)] = ps_av_pool.tile([DK + NH, 512], FP32, name="ps_av")
        ps_av = av_tiles[(c, h)]
        for sl, j in enumerate((2 * jp, 2 * jp + 1)):
            m = j - 4 * c
            off = 0 if m < 0 else m * P
            base = sl * 512
            nc.tensor.matmul(
                ps_av[:, off:512],
                lhsT=vt[:, j, h, :],
                rhs=pt2[:, base : base + 512 - off],
                start=(j == 0),
                stop=(j == njt - 1),
                skip_group_check=True,
            )
        if jp == njt // 2 - 1:
            finish_head(c, h, av_tiles.pop((c, h)))

    # ---- prologue: qkv projections for chunk 0, emitted densely ----
    for g in (
        gen_qkproj(wk_sb, kT, 0),
        gen_vproj(0),
        gen_qkproj(wq_sb, qT, 0),
    ):
        for _ in g:
            pass

    # ---- main loop: S/exp/AV slots with interleaved filler ----
    av_q = deque()
    drained_total = [0]
    deadline = [0]  # fillers that must drain before the next chunk starts

    def drain_tracked(n):
        before = steps_left[0]
        drain(n)
        drained_total[0] += before - steps_left[0]

    for c in range(NT512):
        if c < NT512 - 1:
            push_filler(gen_qkproj(wk_sb, kT, c + 1), 4 * DCH)
            push_filler(gen_vproj(c + 1), 4 * DCH)
            push_filler(gen_qkproj(wq_sb, qT, c + 1), 4 * DCH)
            deadline[0] = drained_total[0] + steps_left[0]
        if c == 2:
            push_filler(gen_outproj(0), 32)
        if c == 3:
            push_filler(gen_outproj(1), 32)
        njt = 4 * (c + 1)
        for h in range(NH):
            for jp in range(njt // 2):
                pt2 = emit_front(c, h, jp, njt)
                av_q.append((c, h, jp, njt, pt2))
                if len(av_q) > AV_LAG:
                    emit_av(*av_q.popleft())
                drain_tracked(DRAIN_BUDGET[c])
        # make sure next chunk's projections finished draining
        while drained_total[0] < deadline[0]:
            drain_tracked(4)

    while av_q:
        emit_av(*av_q.popleft())
    while filler_q:
        drain_tracked(8)


def _build_nc():
    from contextlib import ExitStack

    nc = bacc.Bacc("TRN2", target_bir_lowering=False, debug=False)
    xT = nc.dram_tensor("xT", [D, T], BF16, kind="ExternalInput")
    wq = nc.dram_tensor("wq", [D, E], BF16, kind="ExternalInput")
    wk = nc.dram_tensor("wk", [D, E], BF16, kind="ExternalInput")
    wv = nc.dram_tensor("wv", [D, E], BF16, kind="ExternalInput")
    wo = nc.dram_tensor("wo", [E, D], BF16, kind="ExternalInput")
    mtri = nc.dram_tensor("mtri", [P, P], FP32, kind="ExternalInput")
    sel = nc.dram_tensor("sel", [NH, NH * DK], FP16, kind="ExternalInput")
    out = nc.dram_tensor("out", [T, D], FP32, kind="ExternalOutput")

    with (
        tile.TileContext(nc) as tc,
        nc.allow_low_precision(reason="f16/bf16 intermediates by design"),
        ExitStack() as ctx,
    ):
        _emit(
            nc, tc, ctx, xT[:], wq[:], wk[:], wv[:], wo[:], mtri[:], sel[:], out[:]
        )
    nc.compile()
    return nc


def _host_inputs(x, W_qkv, W_o):
    x = np.asarray(x, dtype=np.float32)
    W_qkv = np.asarray(W_qkv, dtype=np.float32)
    W_o = np.asarray(W_o, dtype=np.float32)
    mtri = np.tril(np.full((P, P), MASK_VAL, dtype=np.float32), -1)
    sel = np.zeros((NH, NH * DK), dtype=np.float16)
    for hh in range(NH):
        sel[hh, hh * DK : (hh + 1) * DK] = 1.0
    bf = ml_dtypes.bfloat16
    in_maps = []
    for b in range(B):
        xTb = np.ascontiguousarray(x[b].T.astype(bf))
        for hg in range(2):
            sl = slice(E * hg, E * hg + E)
            in_maps.append(
                {
                    "xT": xTb,
                    "wq": np.ascontiguousarray(W_qkv[0 * D :][sl].T.astype(bf)),
                    "wk": np.ascontiguousarray(W_qkv[1 * D :][sl].T.astype(bf)),
                    "wv": np.ascontiguousarray(W_qkv[2 * D :][sl].T.astype(bf)),
                    "wo": np.ascontiguousarray(
                        W_o[:, sl].T.astype(ml_dtypes.bfloat16)
                    ),
                    "mtri": mtri,
                    "sel": sel,
                }
            )
    return in_maps


def _run(x, W_qkv, W_o, trace=False, tmpdir=None):
    nc = _build_nc()
    in_maps = _host_inputs(x, W_qkv, W_o)
    res = run_bass_kernel_spmd(
        nc, in_maps, list(range(NCORES)), trace=trace, tmpdir=tmpdir
    )
    out = np.empty((B, T, D), dtype=np.float32)
    for b in range(B):
        out[b] = res.results[2 * b]["out"] + res.results[2 * b + 1]["out"]
    return out, res.exec_time_ns


def kernel(x, W_qkv, W_o):
    out, _ = _run(x, W_qkv, W_o, trace=False)
    return out
